# revision 1
# baseline (speedup 1.0000x reference)
"""Causal self-attention (RoPE + sqk scaling + L2-normalized output) on 8
Trainium2 NeuronCores.

Sharding: tensor-parallel over heads (2 heads/core) for QKV projections and
attention; AllToAll swaps head-sharding for token-sharding; each core then
runs the full output projection + row L2-normalization for its 512-token
slice.

Numerics: projections and Q.K^T scores run in fp32r (full PE rate, ~1.5e-4
rel err); everything downstream of the softmax (P, V, Wo) runs in bf16 with
fp32 PSUM accumulation. RoPE tables carry sqk * sqrt(dim) * 128^(1/4) folded
in, so scores come out of the matmul fully scaled.
"""
import numpy as np
import ml_dtypes

import concourse.bass as bass
import concourse.mybir as mybir
import concourse.tile as tile
from concourse import bacc
from concourse.bass_utils import run_bass_kernel_spmd

# Problem shape (hardcoded per contract).
B, T, DIM = 2, 2048, 2048
N_HEADS, HEAD_DIM = 16, 128
NCORES = 8
HPC = N_HEADS // NCORES          # heads per core = 2
CPC = HPC * HEAD_DIM             # channels per core = 256
NTOK = B * T                     # 4096
TOKS_PER_CORE = NTOK // NCORES   # 512
RESTORE_SCALE = DIM ** 0.5
SC = HEAD_DIM ** 0.25            # each of q,k carries sqrt(score_scale)

F32, F32R, BF16 = mybir.dt.float32, mybir.dt.float32r, mybir.dt.bfloat16
FCH = DIM // 128                 # 16 contraction chunks
TT = 512                         # token tile for projections
NTT = NTOK // TT                 # 8 token tiles
NEG = -1.0e9


def _build_module():
    nc = bacc.Bacc(num_devices=NCORES)

    xt_d = nc.dram_tensor("xt", [DIM, NTOK], F32R, kind="ExternalInput")
    wq_d = nc.dram_tensor("wq", [DIM, CPC], F32R, kind="ExternalInput")
    wk_d = nc.dram_tensor("wk", [DIM, CPC], F32R, kind="ExternalInput")
    wv_d = nc.dram_tensor("wv", [DIM, CPC], F32R, kind="ExternalInput")
    wo_d = nc.dram_tensor("wo", [DIM, DIM], BF16, kind="ExternalInput")
    tab_d = nc.dram_tensor("tabs", [128, 4, T], F32, kind="ExternalInput")
    mask_d = nc.dram_tensor("mask", [128, 128], F32, kind="ExternalInput")
    id_d = nc.dram_tensor("ident", [128, 128], BF16, kind="ExternalInput")
    y_d = nc.dram_tensor("y", [TOKS_PER_CORE, DIM], F32, kind="ExternalOutput")

    a2a_in = nc.dram_tensor("a2a_in", [NCORES, CPC, TOKS_PER_CORE], BF16)
    a2a_out = nc.dram_tensor("a2a_out", [NCORES, CPC, TOKS_PER_CORE], BF16)

    xt_r = xt_d[:].rearrange("(fo p) t -> p fo t", p=128)
    wq_r = wq_d[:].rearrange("(fo p) c -> p fo c", p=128)
    wk_r = wk_d[:].rearrange("(fo p) c -> p fo c", p=128)
    wv_r = wv_d[:].rearrange("(fo p) c -> p fo c", p=128)

    with tile.TileContext(nc) as tc:
        with tc.tile_pool(name="consts", bufs=1) as consts, \
             tc.tile_pool(name="qkv", bufs=1) as qkv:
            mask_t = consts.tile([128, 128], F32)
            nc.sync.dma_start(mask_t[:], mask_d[:])
            ident_t = consts.tile([128, 128], BF16)
            nc.sync.dma_start(ident_t[:], id_d[:])
            eps_t = consts.tile([128, 1], F32)
            nc.vector.memset(eps_t[:], 1e-24)

            # Resident activations.
            QT = qkv.tile([128, HPC, NTOK], F32R)   # [d, h, t]
            KT = qkv.tile([128, HPC, NTOK], F32R)
            Vt = qkv.tile([128, NTOK // 128, CPC], BF16)  # [t%128, tchunk, h*128+d]
            OutT = qkv.tile([128, HPC, NTOK], BF16)  # [d, h, t]

            # ---------------- Phase A: projections + rope ----------------
            with tc.tile_pool(name="wts", bufs=1) as wts, \
                 tc.tile_pool(name="xstream", bufs=2) as xstream, \
                 tc.tile_pool(name="tabs", bufs=1) as tabs, \
                 tc.tile_pool(name="rope", bufs=2) as rope, \
                 tc.tile_pool(name="psA", bufs=1, space="PSUM") as psA, \
                 tc.tile_pool(name="psV", bufs=1, space="PSUM") as psV:
                wq_t = wts.tile([128, FCH, CPC], F32R)
                wk_t = wts.tile([128, FCH, CPC], F32R)
                wv_t = wts.tile([128, FCH, CPC], F32R)
                nc.sync.dma_start(wq_t[:], wq_r)
                nc.sync.dma_start(wk_t[:], wk_r)
                nc.sync.dma_start(wv_t[:], wv_r)

                for tt in range(NTT):
                    tsl = slice(tt * TT, (tt + 1) * TT)
                    tab = tabs.tile([128, 4, TT], F32, tag="tab")
                    psl = (tt * TT) % T
                    nc.sync.dma_start(tab[:], tab_d[:, :, psl:psl + TT])

                    qa = psA.tile([128, TT], F32, tag="qa")
                    qb = psA.tile([128, TT], F32, tag="qb")
                    ka = psA.tile([128, TT], F32, tag="ka")
                    kb = psA.tile([128, TT], F32, tag="kb")
                    vps = []
                    for i in range(4):
                        vtile = psV.tile([128, CPC], F32, tag=f"v{i}", name=f"v{i}")
                        vps.append(vtile[:])

                    for quarter in range(4):
                        xh = xstream.tile([128, 4, TT], F32R, tag="xh")
                        f0 = quarter * 4
                        nc.sync.dma_start(xh[:], xt_r[:, f0:f0 + 4, tsl])
                        for i in range(4):
                            fc = f0 + i
                            st = (fc == 0)
                            sp = (fc == FCH - 1)
                            nc.tensor.matmul(qa[:], wq_t[:, fc, 0:128],
                                             xh[:, i, :], start=st, stop=sp)
                            nc.tensor.matmul(qb[:], wq_t[:, fc, 128:256],
                                             xh[:, i, :], start=st, stop=sp)
                            nc.tensor.matmul(ka[:], wk_t[:, fc, 0:128],
                                             xh[:, i, :], start=st, stop=sp)
                            nc.tensor.matmul(kb[:], wk_t[:, fc, 128:256],
                                             xh[:, i, :], start=st, stop=sp)
                            for ts4 in range(4):
                                nc.tensor.matmul(
                                    vps[ts4],
                                    xh[:, i, ts4 * 128:(ts4 + 1) * 128],
                                    wv_t[:, fc, :], start=st, stop=sp)

                    # drain QK psum via ACT copies, then rope on DVE from SBUF.
                    for name, pa, pb, dst in (("q", qa, qb, QT), ("k", ka, kb, KT)):
                        sa = rope.tile([128, TT], F32, tag="sa")
                        sb_ = rope.tile([128, TT], F32, tag="sb")
                        nc.scalar.copy(sa[:], pa[:])
                        nc.scalar.copy(sb_[:], pb[:])
                        hi = rope.tile([128, TT], F32, tag="hi")
                        t2 = rope.tile([128, TT], F32, tag="t2")
                        nc.vector.tensor_tensor(hi[:], sa[:], tab[:, 2, :],
                                                mybir.AluOpType.mult)
                        nc.vector.tensor_tensor(t2[:], sb_[:], tab[:, 3, :],
                                                mybir.AluOpType.mult)
                        nc.vector.tensor_tensor(hi[:], hi[:], t2[:],
                                                mybir.AluOpType.add)
                        lo = sa  # in-place: sa becomes lo
                        nc.vector.tensor_tensor(lo[:], sa[:], tab[:, 0, :],
                                                mybir.AluOpType.mult)
                        nc.vector.tensor_tensor(t2[:], sb_[:], tab[:, 1, :],
                                                mybir.AluOpType.mult)
                        nc.vector.tensor_tensor(lo[:], lo[:], t2[:],
                                                mybir.AluOpType.subtract)
                        # repack: lo rows 0:64 = head0 dims 0:64; rows 64:128 =
                        # head1 dims 0:64; hi likewise for dims 64:128.
                        lor = lo[:].bitcast(F32R)
                        hir = hi[:].bitcast(F32R)
                        nc.sync.dma_start(dst[0:64, 0, tsl], lor[0:64, :])
                        nc.sync.dma_start(dst[0:64, 1, tsl], lor[64:128, :])
                        nc.sync.dma_start(dst[64:128, 0, tsl], hir[0:64, :])
                        nc.sync.dma_start(dst[64:128, 1, tsl], hir[64:128, :])

                    for ts4 in range(4):
                        nc.scalar.copy(Vt[:, tt * 4 + ts4, :], vps[ts4])

            # ---------------- Phase C: attention ----------------
            QTILES = T // 128        # 16 per batch
            QG = 4                   # q-tiles per group
            with tc.tile_pool(name="s_sb", bufs=3) as s_sbp, \
                 tc.tile_pool(name="p_sb", bufs=3) as p_sbp, \
                 tc.tile_pool(name="ptg", bufs=2) as ptgp, \
                 tc.tile_pool(name="stats", bufs=4) as stats, \
                 tc.tile_pool(name="spsum", bufs=3, space="PSUM") as spsum, \
                 tc.tile_pool(name="ptps", bufs=2, space="PSUM") as ptps, \
                 tc.tile_pool(name="pvps", bufs=2, space="PSUM") as pvps:
                for b in range(B):
                    toff = b * T
                    for h in range(HPC):
                        for qg in range(QTILES // QG):
                            nsc = QG * (qg + 1)  # s-chunks (128) this group
                            ptg = ptgp.tile([128, QTILES, TT], BF16, tag="ptg")
                            for qt in range(QG):
                                qi = qg * QG + qt
                                L = (qi + 1) * 128
                                qsl = slice(toff + qi * 128, toff + qi * 128 + 128)
                                s_row = s_sbp.tile([128, T], F32, tag="s")
                                nblk = qi // QG + 1
                                for sbi in range(nblk):
                                    w = 512 if sbi < qi // QG else (qi % QG + 1) * 128
                                    ps = spsum.tile([128, 512], F32, tag="sps")
                                    nc.tensor.matmul(
                                        ps[:, :w], QT[:, h, qsl],
                                        KT[:, h, toff + sbi * 512: toff + sbi * 512 + w],
                                        start=True, stop=True)
                                    dst_sl = s_row[:, sbi * 512: sbi * 512 + w]
                                    if sbi % 2 == 0:
                                        nc.scalar.copy(dst_sl, ps[:, :w])
                                    else:
                                        nc.vector.tensor_copy(dst_sl, ps[:, :w])
                                # causal mask on the diagonal 128-block
                                dsl = slice(qi * 128, qi * 128 + 128)
                                nc.gpsimd.tensor_tensor(
                                    s_row[:, dsl], s_row[:, dsl], mask_t[:],
                                    mybir.AluOpType.add)
                                mx = stats.tile([128, 1], F32, tag="mx")
                                nc.vector.reduce_max(mx[:], s_row[:, :L],
                                                     axis=mybir.AxisListType.X)
                                nmx = stats.tile([128, 1], F32, tag="nmx")
                                nc.vector.tensor_scalar_mul(nmx[:], mx[:], -1.0)
                                p_row = p_sbp.tile([128, T], BF16, tag="p")
                                sm = stats.tile([128, 1], F32, tag="sm")
                                nc.scalar.activation(
                                    out=s_row[:, :L], in_=s_row[:, :L],
                                    func=mybir.ActivationFunctionType.Exp,
                                    bias=nmx[:], scale=1.0, accum_out=sm[:])
                                nc.gpsimd.normalize_recip(
                                    p_row[:, :L], s_row[:, :L], sm[:])
                                for g4 in range(0, qi + 1, 4):
                                    hi4 = min(g4 + 4, qi + 1)
                                    pt = ptps.tile([128, 1024], BF16, tag="pt")
                                    for sc in range(g4, hi4):
                                        nc.tensor.transpose(
                                            pt[:, (sc - g4) * 128:(sc - g4 + 1) * 128],
                                            p_row[:, sc * 128:(sc + 1) * 128],
                                            ident_t[:])
                                    nc.scalar.copy(
                                        ptg[:, g4:hi4, qt * 128:(qt + 1) * 128],
                                        pt[:, 0:(hi4 - g4) * 128].rearrange(
                                            "p (s q) -> p s q", q=128))
                                for sc in range(qi + 1, nsc):
                                    nc.gpsimd.memset(
                                        ptg[:, sc, qt * 128:(qt + 1) * 128], 0.0)
                            # PV for the group
                            pv = pvps.tile([128, TT], F32, tag="pv")
                            for sc in range(nsc):
                                nc.tensor.matmul(
                                    pv[:], Vt[:, b * 16 + sc, h * 128:(h + 1) * 128],
                                    ptg[:, sc, :],
                                    start=(sc == 0), stop=(sc == nsc - 1))
                            gsl = slice(toff + qg * TT, toff + (qg + 1) * TT)
                            nc.scalar.copy(OutT[:, h, gsl], pv[:])

            # ---------------- Phase D: A2A + output projection ----------------
            for j in range(NCORES):
                nc.sync.dma_start(
                    a2a_in[j].rearrange("(h p) t -> p h t", p=128),
                    OutT[:, :, j * TOKS_PER_CORE:(j + 1) * TOKS_PER_CORE])
            nc.gpsimd.collective_compute(
                "AllToAll", mybir.AluOpType.bypass,
                replica_groups=[list(range(NCORES))],
                ins=[a2a_in[:]], outs=[a2a_out[:]])

            with tc.tile_pool(name="phD", bufs=1) as phD, \
                 tc.tile_pool(name="ysb", bufs=2) as ysb, \
                 tc.tile_pool(name="ydve", bufs=2) as ydve, \
                 tc.tile_pool(name="yps", bufs=2, space="PSUM") as yps:
                ot = phD.tile([128, FCH, TOKS_PER_CORE], BF16)
                nc.sync.dma_start(
                    ot[:], a2a_out[:].rearrange("j (h p) t -> p (j h) t", p=128))
                wo_t = phD.tile([128, FCH, DIM], BF16)
                nc.sync.dma_start(
                    wo_t[:], wo_d[:].rearrange("(cc p) e -> p cc e", p=128))
                for t4 in range(TOKS_PER_CORE // 128):
                    y_row = ysb.tile([128, DIM], F32, tag="y")
                    for et in range(DIM // 512):
                        yp = yps.tile([128, 512], F32, tag="yp")
                        for cc in range(FCH):
                            nc.tensor.matmul(
                                yp[:], ot[:, cc, t4 * 128:(t4 + 1) * 128],
                                wo_t[:, cc, et * 512:(et + 1) * 512],
                                start=(cc == 0), stop=(cc == FCH - 1))
                        nc.scalar.copy(y_row[:, et * 512:(et + 1) * 512], yp[:])
                    sq = ydve.tile([128, DIM], BF16, tag="sq")
                    ss = ydve.tile([128, 1], F32, tag="ss")
                    nc.scalar.activation(
                        out=sq[:], in_=y_row[:],
                        func=mybir.ActivationFunctionType.Square,
                        accum_out=ss[:])
                    nrm = ydve.tile([128, 1], F32, tag="nrm")
                    nc.scalar.activation(
                        out=nrm[:], in_=ss[:],
                        func=mybir.ActivationFunctionType.Sqrt,
                        bias=eps_t[:], scale=1.0)
                    rn = ydve.tile([128, 1], F32, tag="rn")
                    nc.vector.reciprocal(rn[:], nrm[:])
                    nc.vector.tensor_scalar_mul(y_row[:], y_row[:], rn[:])
                    nc.sync.dma_start(y_d[t4 * 128:(t4 + 1) * 128, :], y_row[:])

    nc.finalize()
    return nc


_NC_CACHE = None


def _get_module():
    global _NC_CACHE
    if _NC_CACHE is None:
        _NC_CACHE = _build_module()
    return _NC_CACHE


def _host_prep(x, Wq, Wk, Wv, Wo, sqk):
    x = np.asarray(x, dtype=np.float32)
    Wq = np.asarray(Wq, dtype=np.float32)
    Wk = np.asarray(Wk, dtype=np.float32)
    Wv = np.asarray(Wv, dtype=np.float32)
    Wo = np.asarray(Wo, dtype=np.float32)
    sqk = np.asarray(sqk, dtype=np.float32)

    xt = np.ascontiguousarray(x.reshape(NTOK, DIM).T)  # [DIM, NTOK]
    wo = np.ascontiguousarray(Wo.T).astype(ml_dtypes.bfloat16)  # [c, e]

    # rope base tables
    pos = np.arange(T, dtype=np.float64)
    theta = 1.0 / (10000.0 ** (np.arange(0, HEAD_DIM, 2, dtype=np.float64) / HEAD_DIM))
    ang = pos[None, :] * theta[:, None]            # [64, T]
    cosb, sinb = np.cos(ang), np.sin(ang)

    s_all = sqk.reshape(N_HEADS, HEAD_DIM).astype(np.float64) * RESTORE_SCALE * SC

    mask = np.where(np.arange(128)[None, :] <= np.arange(128)[:, None],
                    0.0, NEG).astype(np.float32)
    ident = np.eye(128, dtype=np.float32).astype(ml_dtypes.bfloat16)

    in_maps = []
    for c in range(NCORES):
        h0, h1 = HPC * c, HPC * c + 1
        rows = []
        for half in (slice(0, 64), slice(64, 128)):
            rows.extend(range(h0 * HEAD_DIM + half.start, h0 * HEAD_DIM + half.stop))
            rows.extend(range(h1 * HEAD_DIM + half.start, h1 * HEAD_DIM + half.stop))
        rows = np.array(rows)
        wq = np.ascontiguousarray(Wq[rows, :].T)   # [DIM, 256] cols: lo|hi
        wk = np.ascontiguousarray(Wk[rows, :].T)
        vrows = np.arange(h0 * HEAD_DIM, (h1 + 1) * HEAD_DIM)
        wv = np.ascontiguousarray(Wv[vrows, :].T)  # [DIM, 256] head-major

        # tables [128, 4, T]: partition p<64 -> head0 freq p; p>=64 -> head1.
        tabs = np.empty((128, 4, T), dtype=np.float32)
        for j, h in ((0, h0), (64, h1)):
            s_lo = s_all[h, 0:64][:, None]         # scale for output dims 0:64
            s_hi = s_all[h, 64:128][:, None]
            tabs[j:j + 64, 0, :] = cosb * s_lo     # cosL
            tabs[j:j + 64, 1, :] = sinb * s_lo     # sinL
            tabs[j:j + 64, 2, :] = sinb * s_hi     # sinH
            tabs[j:j + 64, 3, :] = cosb * s_hi     # cosH

        in_maps.append({
            "xt": xt, "wq": wq, "wk": wk, "wv": wv, "wo": wo,
            "tabs": tabs, "mask": mask, "ident": ident,
        })
    return in_maps


def kernel(x, Wq, Wk, Wv, Wo, sqk):
    nc = _get_module()
    in_maps = _host_prep(x, Wq, Wk, Wv, Wo, sqk)
    res = run_bass_kernel_spmd(nc, in_maps, core_ids=list(range(NCORES)))
    y = np.concatenate([res.results[c]["y"] for c in range(NCORES)], axis=0)
    return y.reshape(B, T, DIM)



# revision 19
# speedup vs baseline: 1.4932x; 1.4932x over previous
"""Causal self-attention (RoPE + sqk scaling + L2-normalized output) on 8
Trainium2 NeuronCores.

Sharding: tensor-parallel over heads (2 heads/core). AllToAll (4 quarter
chunks, pipelined with attention stripes and the output projection) swaps
head-sharding for token-sharding; each core runs the output projection +
L2-norm for its 512-token slice.

PE program order:
  A-seg1: QKV projections for tokens 0..1791 of both batches
  stripe0: attention q-tiles qi%4==0  -> A2A chunk 0
  A-seg2: remaining projection tiles
  stripe1, stripe3, stripe2 -> A2A chunks 1,3,2
  D0, D1, D3, D2: o-proj + L2 norm per 128-token quarter of my slice

Softmax: global per-row max via bf16-view PSUM reads (DVE), exp on ACT with
accumulated row-sum, 1/l applied in bf16 by DVE+Pool, P^T produced by
DMA-engine transposes (dma_start_transpose) straight to SBUF.
"""
import numpy as np
import ml_dtypes

import concourse.mybir as mybir
import concourse.tile as tile
from concourse import bacc
from concourse.bass_utils import run_bass_kernel_spmd

B, T, DIM = 2, 2048, 2048
N_HEADS, HEAD_DIM = 16, 128
NCORES = 8
HPC = N_HEADS // NCORES          # heads per core = 2
CPC = HPC * HEAD_DIM             # channels per core = 256
NTOK = B * T                     # 4096
TPC = NTOK // NCORES             # tokens per core = 512
RESTORE_SCALE = DIM ** 0.5
SC = HEAD_DIM ** 0.25            # each of q,k carries sqrt(score_scale)

F32, F32R, BF16 = mybir.dt.float32, mybir.dt.float32r, mybir.dt.bfloat16
FCH = DIM // 128                 # 16 contraction chunks
TT = 256                         # token tile for projections
NEG = -1.0e9
DEBUG_DUMP = False

# phase A tile groups (tt: 0-7 = batch0, 8-15 = batch1; 256 tokens each).
# seg1 covers tokens 0..1791 of each batch (enough for stripe0's qi<=12).
SEG1 = [[0, 1], [8, 9], [2, 3], [10, 11], [4, 5], [12, 13], [6], [14]]
SEG2 = [[7], [15]]


def _score_widths(L):
    """PSUM bank widths covering L score columns; avoid 128-wide fp32r
    blocks (fp32r matmul needs free>=256 for full rate)."""
    nfull, rem = divmod(L, 512)
    w = [512] * nfull
    if rem == 128 and nfull >= 1:
        w = w[:-1] + [384, 256]
    elif rem:
        w.append(rem)
    return w


def _build_module():
    nc = bacc.Bacc(num_devices=NCORES)

    xt_d = nc.dram_tensor("xt", [DIM, NTOK], F32R, kind="ExternalInput")
    wq_d = nc.dram_tensor("wq", [DIM, CPC], F32R, kind="ExternalInput")
    wk_d = nc.dram_tensor("wk", [DIM, CPC], F32R, kind="ExternalInput")
    wv_d = nc.dram_tensor("wv", [DIM, CPC], F32R, kind="ExternalInput")
    wo_d = nc.dram_tensor("wo", [DIM, DIM], BF16, kind="ExternalInput")
    tab_d = nc.dram_tensor("tabs", [128, 4, T], F32, kind="ExternalInput")
    mask_d = nc.dram_tensor("mask", [128, 128], F32, kind="ExternalInput")
    id_d = nc.dram_tensor("ident", [128, 128], BF16, kind="ExternalInput")
    y_d = nc.dram_tensor("y", [TPC, DIM], F32, kind="ExternalOutput")

    dbga_d = nc.dram_tensor("dbga", [NCORES, CPC, 128], BF16, kind="ExternalOutput")
    dbgb_d = nc.dram_tensor("dbgb", [NCORES, CPC, 128], BF16, kind="ExternalOutput")
    dbgq_d = nc.dram_tensor("dbgq", [128, HPC, NTOK], F32, kind="ExternalOutput")
    dbgk_d = nc.dram_tensor("dbgk", [128, HPC, NTOK], F32, kind="ExternalOutput")
    dbgv_d = nc.dram_tensor("dbgv", [128, NTOK // 128, CPC], F32, kind="ExternalOutput")
    a2a_in_all = nc.dram_tensor("a2a_in", [4, NCORES, CPC, 128], BF16)
    a2a_out_all = nc.dram_tensor("a2a_out", [4, NCORES, CPC, 128], BF16)
    a2a_in = [a2a_in_all[q] for q in range(4)]
    a2a_out = [a2a_out_all[q] for q in range(4)]

    xt_r = xt_d[:].rearrange("(fo p) t -> p fo t", p=128)
    w_srcs = {"wq": wq_d, "wk": wk_d, "wv": wv_d}

    with tile.TileContext(nc) as tc:
        with tc.tile_pool(name="consts", bufs=1) as consts, \
             tc.tile_pool(name="qkv", bufs=1) as qkv:
            # Resident activations.
            QT = qkv.tile([128, HPC, NTOK], F32R)   # [d, h, t]
            KT = qkv.tile([128, HPC, NTOK], F32R)
            Vt = qkv.tile([128, NTOK // 128, CPC], BF16)  # [t%128, tchunk, ch]

            mask_t = consts.tile([128, 128], F32)
            ident_t = consts.tile([128, 128], BF16)
            eps_t = consts.tile([128, 1], F32)
            nc.vector.memset(eps_t[:], 1e-24)

            # ------------- phase A -------------
            def run_a_group(group, w_tiles, tab_t, xstream, ropep, psA, psV):
                glen = len(group) * TT
                Lq = ropep.tile([128, 2, 512], F32, tag="lq", name="Lq")
                Lk = ropep.tile([128, 2, 512], F32, tag="lk", name="Lk")
                for gi, tt in enumerate(group):
                    tsl = slice(tt * TT, (tt + 1) * TT)
                    qab = psA.tile([128, 2 * TT], F32, tag="qab", name="qab")
                    kab = psA.tile([128, 2 * TT], F32, tag="kab", name="kab")
                    vps = psV.tile([128, 2 * TT], F32, tag="vps", name="vps")
                    xhq = []
                    for quarter in range(4):
                        xh = xstream.tile([128, 4, TT], F32R, tag="xh",
                                          name="xh")
                        nc.scalar.dma_start(
                            xh[:], xt_r[:, quarter * 4:quarter * 4 + 4, tsl])
                        xhq.append(xh)

                    def mm(dst, lhsT, rhs, fc):
                        nc.tensor.matmul(dst, lhsT, rhs,
                                         start=(fc == 0), stop=(fc == FCH - 1))

                    # one accumulation chain at a time per PSUM bank
                    # (interleaved chains in a bank lose the first chain's
                    # fc0 contribution - group semantics are per-bank).
                    for name, dst, col in (("wq", qab, 0), ("wq", qab, 1),
                                           ("wk", kab, 0), ("wk", kab, 1)):
                        for fc in range(FCH):
                            wt = w_tiles[(name, fc // 8)]
                            mm(dst[:, col * TT:(col + 1) * TT],
                               wt[:, fc % 8, col * 128:(col + 1) * 128],
                               xhq[fc // 4][:, fc % 4, :], fc)
                    for grp in (0, 1):
                        for fc in range(FCH):
                            wt = w_tiles[("wv", fc // 8)]
                            mm(vps[:, grp * TT:(grp + 1) * TT],
                               xhq[fc // 4][:, fc % 4, grp * 128:(grp + 1) * 128],
                               wt[:, fc % 8, :], fc)

                    # rope (DVE, reads PSUM) into the pair tile.
                    pos = tt % 8
                    tabg = tab_t[pos // 2][:, :,
                                           (pos % 2) * 256:(pos % 2) * 256 + 256]
                    off = gi * 256
                    for Lt, ab in ((Lq, qab), (Lk, kab)):
                        sa, sb = ab[:, 0:TT], ab[:, TT:2 * TT]
                        t2 = ropep.tile([128, TT], F32, tag="t2", name="t2")
                        lo = Lt[:, 0, off:off + 256]
                        hi = Lt[:, 1, off:off + 256]
                        nc.vector.tensor_tensor(hi, sa, tabg[:, 2, :],
                                                mybir.AluOpType.mult)
                        nc.vector.tensor_tensor(t2[:], sb, tabg[:, 3, :],
                                                mybir.AluOpType.mult)
                        nc.vector.tensor_tensor(hi, hi, t2[:],
                                                mybir.AluOpType.add)
                        nc.vector.tensor_tensor(lo, sa, tabg[:, 0, :],
                                                mybir.AluOpType.mult)
                        nc.vector.tensor_tensor(t2[:], sb, tabg[:, 1, :],
                                                mybir.AluOpType.mult)
                        nc.vector.tensor_tensor(lo, lo, t2[:],
                                                mybir.AluOpType.subtract)
                    # V drain (ACT) f32 -> bf16
                    nc.scalar.copy(Vt[:, tt * 2, :], vps[:, 0:TT])
                    nc.scalar.copy(Vt[:, tt * 2 + 1, :], vps[:, TT:])

                # repack pair/solo -> QT/KT (SP queue, 64-partition DMAs)
                gsl = slice(group[0] * TT, group[0] * TT + glen)
                for Lt, dst in ((Lq, QT), (Lk, KT)):
                    Lr = Lt[:].bitcast(F32R)
                    for h in range(2):
                        nc.sync.dma_start(dst[0:64, h, gsl],
                                          Lr[h * 64:h * 64 + 64, 0, 0:glen])
                        nc.sync.dma_start(dst[64:128, h, gsl],
                                          Lr[h * 64:h * 64 + 64, 1, 0:glen])

            # ------------- phase C -------------
            def make_stripe(q):
                out = []
                for b in range(B):
                    for h in range(HPC):
                        for jj in range(4):
                            out.append((b, h, 4 * jj + q, jj))
                return out

            def run_stripe(items, chunk_q, spsum, accp, prow, ptgp, statp,
                           stgp, ptps):
                stg = {}
                pending = []

                def do_pv(ent):
                    (b, h, qi, jj, ptg) = ent
                    pv = accp.tile([128, 128], F32, tag="acc", name="pv")
                    for sc in range(qi + 1):
                        nc.tensor.matmul(
                            pv[:], Vt[:, b * 16 + sc, h * 128:(h + 1) * 128],
                            ptg[:, sc, :],
                            start=(sc == 0), stop=(sc == qi))
                    nc.scalar.copy(stg[(b, h)][:, jj, :], pv[:])

                for (b, h, qi, jj) in items:
                    if (b, h) not in stg:
                        stg[(b, h)] = stgp.tile([128, 4, 128], BF16,
                                                tag="stg", name="stg")
                    toff = b * T
                    L = (qi + 1) * 128
                    qsl = slice(toff + qi * 128, toff + qi * 128 + 128)
                    nblk = (L + 511) // 512
                    rem = L - (nblk - 1) * 512
                    nmx = statp.tile([128, 4], F32, tag="nmx", name="nmx")
                    l4 = statp.tile([128, 4], F32, tag="l4", name="l4")
                    stiles = []
                    for bi in range(nblk):
                        off = bi * 512
                        ps = spsum.tile([128, 512], F32, tag="s", name="ps")
                        stiles.append((ps, off, 512))
                        nc.tensor.matmul(
                            ps[:], QT[:, h, qsl],
                            KT[:, h, toff + off: toff + off + 512],
                            start=True, stop=True)
                    # pad region beyond L and causal diagonal get -1e9
                    pl, po, pw = stiles[-1]
                    if rem < 512:
                        nc.vector.memset(pl[:, rem:512], NEG)
                    nc.vector.tensor_tensor(pl[:, rem - 128:rem],
                                            pl[:, rem - 128:rem], mask_t[:],
                                            mybir.AluOpType.add)
                    p_row = prow.tile([128, (jj + 1) * 512], BF16,
                                      tag=f"p{jj}", bufs=2, name="p_row")
                    for bi, (ps, o_, w) in enumerate(stiles):
                        nc.vector.reduce_max(nmx[:, bi:bi + 1], ps[:],
                                             axis=mybir.AxisListType.X,
                                             negate=True)
                        nc.scalar.activation(
                            out=p_row[:, o_:o_ + w], in_=ps[:],
                            func=mybir.ActivationFunctionType.Exp,
                            bias=nmx[:, bi:bi + 1], scale=1.0,
                            accum_out=l4[:, bi:bi + 1])
                    nm = statp.tile([128, 1], F32, tag="nm", name="nm")
                    nc.vector.tensor_reduce(nm[:], nmx[:, 0:nblk],
                                            axis=mybir.AxisListType.X,
                                            op=mybir.AluOpType.min)
                    e4 = statp.tile([128, 4], F32, tag="e4", name="e4")
                    nc.scalar.activation(out=e4[:, 0:nblk], in_=nmx[:, 0:nblk],
                                         func=mybir.ActivationFunctionType.Exp,
                                         bias=nm[:], scale=-1.0)
                    le4 = statp.tile([128, 4], F32, tag="le4", name="le4")
                    nc.vector.tensor_tensor(le4[:, 0:nblk], l4[:, 0:nblk],
                                            e4[:, 0:nblk],
                                            mybir.AluOpType.mult)
                    lt = statp.tile([128, 1], F32, tag="lt", name="lt")
                    nc.vector.tensor_reduce(lt[:], le4[:, 0:nblk],
                                            axis=mybir.AxisListType.X,
                                            op=mybir.AluOpType.add)
                    r = statp.tile([128, 1], F32, tag="r", name="r")
                    nc.vector.reciprocal(r[:], lt[:])
                    re4 = statp.tile([128, 4], F32, tag="re4", name="re4")
                    nc.vector.tensor_scalar_mul(re4[:, 0:nblk], e4[:, 0:nblk],
                                                r[:])
                    for bi, (ps, o_, w) in enumerate(stiles):
                        eng = nc.vector if bi % 2 == 0 else nc.gpsimd
                        eng.tensor_scalar_mul(p_row[:, o_:o_ + w],
                                              p_row[:, o_:o_ + w],
                                              re4[:, bi:bi + 1])
                    ptg = ptgp.tile([128, 4 * (jj + 1), 128], BF16,
                                    tag=f"ptg{jj}", bufs=2, name="ptg")
                    for g4 in range(0, qi + 1, 4):
                        hi4 = min(g4 + 4, qi + 1)
                        pt = ptps.tile([128, 4, 128], BF16, tag="pt",
                                       name="pt")
                        for sc in range(g4, hi4):
                            nc.tensor.transpose(
                                pt[:, sc - g4, :],
                                p_row[:, sc * 128:(sc + 1) * 128],
                                ident_t[:])
                        nc.scalar.copy(ptg[:, g4:hi4, :],
                                       pt[:, 0:hi4 - g4, :])
                    pending.append((b, h, qi, jj, ptg))
                    if len(pending) > 6:
                        do_pv(pending.pop(0))
                while pending:
                    do_pv(pending.pop(0))
                for (b, h), st in stg.items():
                    dst = a2a_in[chunk_q][b * 4:(b + 1) * 4,
                                             h * 128:(h + 1) * 128, :]
                    nc.scalar.dma_start(dst.rearrange("j p t -> p j t"),
                                        st[:])
                nc.gpsimd.collective_compute(
                    "AllToAll", mybir.AluOpType.bypass,
                    replica_groups=[list(range(NCORES))],
                    ins=[a2a_in[chunk_q]], outs=[a2a_out[chunk_q]])

            # ------------- phase D -------------
            def run_d(q, wo_t, accp, otp, ysb):
                ot = otp.tile([128, FCH, 128], BF16, tag="ot", name="ot")
                nc.sync.dma_start(
                    ot[:], a2a_out[q].rearrange("j (h p) t -> p (j h) t",
                                                   p=128))
                y_row = ysb.tile([128, DIM], F32, tag="y", name="y_row")
                for et in range(4):
                    yp = accp.tile([128, 512], F32, tag="acc", name="yp")
                    for cc in range(FCH):
                        nc.tensor.matmul(
                            yp[:], ot[:, cc, :],
                            wo_t[:, cc, et * 512:(et + 1) * 512],
                            start=(cc == 0), stop=(cc == FCH - 1))
                    dsl = y_row[:, et * 512:(et + 1) * 512]
                    if et % 2 == 0:
                        nc.scalar.copy(dsl, yp[:])
                    else:
                        nc.vector.tensor_copy(dsl, yp[:])
                sq = ysb.tile([128, DIM], BF16, tag="sq", bufs=1, name="sq")
                ss = ysb.tile([128, 1], F32, tag="ss", name="ss")
                nc.scalar.activation(out=sq[:], in_=y_row[:],
                                     func=mybir.ActivationFunctionType.Square,
                                     accum_out=ss[:])
                nrm = ysb.tile([128, 1], F32, tag="nrm", name="nrm")
                nc.scalar.activation(out=nrm[:], in_=ss[:],
                                     func=mybir.ActivationFunctionType.Sqrt,
                                     bias=eps_t[:], scale=1.0)
                rn = ysb.tile([128, 1], F32, tag="rn", name="rn")
                nc.vector.reciprocal(rn[:], nrm[:])
                nc.vector.tensor_scalar_mul(y_row[:], y_row[:], rn[:])
                nc.scalar.dma_start(y_d[q * 128:(q + 1) * 128, :], y_row[:])

            # ------------- program -------------
            with tc.tile_pool(name="wts", bufs=1) as wts, \
                 tc.tile_pool(name="tabsp", bufs=1) as tabsp:
                w_tiles = {}
                for name in ("wq", "wk", "wv"):
                    src = w_srcs[name][:].rearrange("(fo p) c -> p fo c",
                                                    p=128)
                    for half in (0, 1):
                        t = wts.tile([128, 8, CPC], F32R,
                                     name=f"{name}{half}")
                        nc.sync.dma_start(t[:],
                                          src[:, half * 8:(half + 1) * 8, :])
                        w_tiles[(name, half)] = t
                tab_t = []
                for g in range(4):
                    t = tabsp.tile([128, 4, 512], F32, name=f"tab{g}")
                    nc.sync.dma_start(t[:], tab_d[:, :, g * 512:(g + 1) * 512])
                    tab_t.append(t)
                nc.sync.dma_start(mask_t[:], mask_d[:])
                nc.sync.dma_start(ident_t[:], id_d[:])

                with tc.tile_pool(name="xs1", bufs=7) as xstream, \
                     tc.tile_pool(name="rp1", bufs=2) as ropep, \
                     tc.tile_pool(name="psA1", bufs=2, space="PSUM") as psA, \
                     tc.tile_pool(name="psV1", bufs=2, space="PSUM") as psV:
                    for group in SEG1:
                        run_a_group(group, w_tiles, tab_t, xstream, ropep,
                                    psA, psV)

                with tc.tile_pool(name="sps0", bufs=4, space="PSUM") as sps, \
                     tc.tile_pool(name="ptps0", bufs=2, space="PSUM") as ptps, \
                     tc.tile_pool(name="acc0", bufs=2, space="PSUM") as accp, \
                     tc.tile_pool(name="prow0", bufs=3) as prow, \
                     tc.tile_pool(name="ptgp0", bufs=3) as ptgp, \
                     tc.tile_pool(name="statp0", bufs=8) as statp, \
                     tc.tile_pool(name="stgp0", bufs=5) as stgp:
                    run_stripe(make_stripe(0), 0, sps, accp, prow, ptgp,
                               statp, stgp, ptps)

                with tc.tile_pool(name="xs2", bufs=7) as xstream, \
                     tc.tile_pool(name="rp2", bufs=2) as ropep, \
                     tc.tile_pool(name="psA2", bufs=2, space="PSUM") as psA, \
                     tc.tile_pool(name="psV2", bufs=2, space="PSUM") as psV:
                    for group in SEG2:
                        run_a_group(group, w_tiles, tab_t, xstream, ropep,
                                    psA, psV)

            # weights/tabs pools closed; load Wo into the freed space.
            with tc.tile_pool(name="wop", bufs=1) as wop:
                wo_t = wop.tile([128, FCH, DIM], BF16, name="wo_t")
                wo_r = wo_d[:].rearrange("(cc p) e -> p cc e", p=128)
                for ch in range(4):
                    nc.sync.dma_start(wo_t[:, ch * 4:(ch + 1) * 4, :],
                                      wo_r[:, ch * 4:(ch + 1) * 4, :])
                with tc.tile_pool(name="spsC", bufs=4, space="PSUM") as sps, \
                     tc.tile_pool(name="ptpsC", bufs=2, space="PSUM") as ptps, \
                     tc.tile_pool(name="accC", bufs=2, space="PSUM") as accp, \
                     tc.tile_pool(name="prowC", bufs=2) as prow, \
                     tc.tile_pool(name="ptgpC", bufs=2) as ptgp, \
                     tc.tile_pool(name="statpC", bufs=8) as statp, \
                     tc.tile_pool(name="stgpC", bufs=5) as stgp:
                    for q in (1, 3, 2):
                        run_stripe(make_stripe(q), q, sps, accp, prow, ptgp,
                                   statp, stgp, ptps)
                with tc.tile_pool(name="accD", bufs=2, space="PSUM") as accp, \
                     tc.tile_pool(name="otp", bufs=2) as otp, \
                     tc.tile_pool(name="ysb", bufs=2) as ysb:
                    for q in (0, 1, 3, 2):
                        run_d(q, wo_t, accp, otp, ysb)
                    if DEBUG_DUMP:
                        nc.gpsimd.dma_start(dbga_d[:], a2a_in[1])
                        nc.gpsimd.dma_start(dbgb_d[:], a2a_in[0])
                        nc.gpsimd.dma_start(dbgq_d[:], QT[:].bitcast(F32))
                        nc.gpsimd.dma_start(dbgk_d[:], KT[:].bitcast(F32))
                        nc.gpsimd.dma_start(dbgv_d[:], Vt[:])

    nc.finalize()
    return nc


_NC_CACHE = None


def _get_module():
    global _NC_CACHE
    if _NC_CACHE is None:
        _NC_CACHE = _build_module()
    return _NC_CACHE


def _host_prep(x, Wq, Wk, Wv, Wo, sqk):
    x = np.asarray(x, dtype=np.float32)
    Wq = np.asarray(Wq, dtype=np.float32)
    Wk = np.asarray(Wk, dtype=np.float32)
    Wv = np.asarray(Wv, dtype=np.float32)
    Wo = np.asarray(Wo, dtype=np.float32)
    sqk = np.asarray(sqk, dtype=np.float32)

    xt = np.ascontiguousarray(x.reshape(NTOK, DIM).T)  # [DIM, NTOK]
    wo = np.ascontiguousarray(Wo.T).astype(ml_dtypes.bfloat16)  # [c, e]

    pos = np.arange(T, dtype=np.float64)
    theta = 1.0 / (10000.0 ** (np.arange(0, HEAD_DIM, 2,
                                         dtype=np.float64) / HEAD_DIM))
    ang = pos[None, :] * theta[:, None]            # [64, T]
    cosb, sinb = np.cos(ang), np.sin(ang)

    s_all = sqk.reshape(N_HEADS, HEAD_DIM).astype(np.float64) \
        * RESTORE_SCALE * SC

    mask = np.where(np.arange(128)[None, :] <= np.arange(128)[:, None],
                    0.0, NEG).astype(np.float32)
    ident = np.eye(128, dtype=np.float32).astype(ml_dtypes.bfloat16)

    in_maps = []
    for c in range(NCORES):
        h0, h1 = HPC * c, HPC * c + 1
        rows = []
        for half in (slice(0, 64), slice(64, 128)):
            rows.extend(range(h0 * HEAD_DIM + half.start,
                              h0 * HEAD_DIM + half.stop))
            rows.extend(range(h1 * HEAD_DIM + half.start,
                              h1 * HEAD_DIM + half.stop))
        rows = np.array(rows)
        wq = np.ascontiguousarray(Wq[rows, :].T)   # [DIM, 256] cols: lo|hi
        wk = np.ascontiguousarray(Wk[rows, :].T)
        vrows = np.arange(h0 * HEAD_DIM, (h1 + 1) * HEAD_DIM)
        wv = np.ascontiguousarray(Wv[vrows, :].T)  # [DIM, 256] head-major

        tabs = np.empty((128, 4, T), dtype=np.float32)
        for j, h in ((0, h0), (64, h1)):
            s_lo = s_all[h, 0:64][:, None]
            s_hi = s_all[h, 64:128][:, None]
            tabs[j:j + 64, 0, :] = cosb * s_lo     # cosL
            tabs[j:j + 64, 1, :] = sinb * s_lo     # sinL
            tabs[j:j + 64, 2, :] = sinb * s_hi     # sinH
            tabs[j:j + 64, 3, :] = cosb * s_hi     # cosH

        in_maps.append({
            "xt": xt, "wq": wq, "wk": wk, "wv": wv, "wo": wo,
            "tabs": tabs, "mask": mask, "ident": ident,
        })
    return in_maps


def kernel(x, Wq, Wk, Wv, Wo, sqk):
    nc = _get_module()
    in_maps = _host_prep(x, Wq, Wk, Wv, Wo, sqk)
    res = run_bass_kernel_spmd(nc, in_maps, core_ids=list(range(NCORES)))
    y = np.concatenate([res.results[c]["y"] for c in range(NCORES)], axis=0)
    return y.reshape(B, T, DIM)
